# revision 39
# baseline (speedup 1.0000x reference)
"""AffEncoder Trainium2 kernel.

The network folds into 4 temporal-conv stages (channel-major):
  s1: K=28  (27 pose ch + ones row), M=144, 9 taps   (conv1 + A1 einsum folded)
  s2: K=145 (144 ch + ones row),     M=48,  9 taps   (gather + conv2 + A2 folded)
  s3: K=48, M=16, 5 taps, then Lrelu(scale*x+bias)   (convc1 + bn1 folded)
  s4: K=16, M=8,  3 taps, then Lrelu(scale*x+bias)   (convc2 + bn2 folded)

Sharding: pure data parallel, 32 batch elements per core across 8 cores.
Host does the (n,t,c)->(n,c,t) transposes + weight folding; the device runs
channel-major matmul pipelines.

Per batch element, per 512-col t-tile:
  s1: taps 0-3 / 4-7 pre-shifted into two 112-row operand stacks (one
      windowed DMA each); tap 8 = stack1 block 0 at window +8     -> 6 MM
  s2: 9 full-K passes (ch 0-127) + stacked B pass (ch 128-143 x 8 taps,
      one windowed SBUF->SBUF DMA) + tap-8/bias pass              -> 11 MM
  s3: K=128 double-tap passes on a partition-shifted replica      -> 3 MM
  s4: split K=32 (ACT-written rows) + K=64 (DVE replica rows)     -> 2 MM

TRN2 matmuls accept only ONE sync-wait, so each matmul's operands live in
tiles with a single producer: xs1/xs2/bstk are each written by exactly one
DMA instruction; o1a/o1b/o2s and the o3s replica rows are written only by
DVE; o3s rows 0-31 only by ACT.  Channels are padded (48->64, 16->32) so
partition-shifted replicas start 32-aligned.
"""
import os
import sys
import numpy as np

for _p in ("/opt/trn_rl_repo",):
    if _p not in sys.path and os.path.isdir(_p):
        sys.path.insert(0, _p)

import ml_dtypes  # noqa: E402

N_CORES = 8
N, T = 256, 1024
NPC = N // N_CORES
EPS = 1e-5
J, C, K1, K2, F1, F2 = 9, 3, 5, 3, 16, 16
NUM_PARTS, MAX_EDGES = 3, 3

XW = T + 12                 # x pad: 4 left, 8 right
O2W = T + 6                 # o2s pad: 2 left, 4 right
O3W = T + 4                 # o3s pad: 1 left, 3 right
DTYPE = os.environ.get("BASS_DTYPE", "bf16")  # bf16 | f32r | f32
STAGES = int(os.environ.get("STAGES", "4"))   # debug: truncate pipeline


def fold_weights(W1, b1, A1, W2, b2, A2, Wc1, bc1, bn1_w, bn1_b, bn1_m, bn1_v,
                 Wc2, bc2, bn2_w, bn2_b, bn2_m, bn2_v):
    W1 = np.asarray(W1, np.float64); A1 = np.asarray(A1, np.float64)
    W2 = np.asarray(W2, np.float64); A2 = np.asarray(A2, np.float64)

    W1r = W1[:, :, :, 0].reshape(K1, F1, C, 9)              # [k, c, ci, dt]
    W1t = np.zeros((9, 28, 144))
    W1t[:, :27, :] = np.einsum('kcid,kvw->dvicw', W1r, A1).reshape(9, 27, 144)
    beff1 = np.einsum('kc,kw->cw', np.asarray(b1, np.float64).reshape(K1, F1),
                      A1.sum(axis=1)).reshape(144)
    W1t[4, 27, :] = beff1

    W2r = W2[:, :, :, 0].reshape(K2, F2, F1, MAX_EDGES, 9)  # [k2, c2, c, e, dt]
    W2t = np.zeros((9, 145, 48))
    W2t[:, :144, :] = np.einsum('kbced,kpq->dcpebq', W2r, A2).reshape(9, 144, 48)
    beff2 = np.einsum('kb,kq->bq', np.asarray(b2, np.float64).reshape(K2, F2),
                      A2.sum(axis=1)).reshape(48)
    W2t[4, 144, :] = beff2

    Wc1t = np.asarray(Wc1, np.float64).transpose(2, 1, 0)   # [dt, m2, c3]
    scale3 = np.asarray(bn1_w, np.float64) / np.sqrt(np.asarray(bn1_v, np.float64) + EPS)
    bias3 = scale3 * np.asarray(bc1, np.float64) + (np.asarray(bn1_b, np.float64)
            - np.asarray(bn1_m, np.float64) * scale3)
    Wc2t = np.asarray(Wc2, np.float64).transpose(2, 1, 0)   # [dt, c3, c4]
    scale4 = np.asarray(bn2_w, np.float64) / np.sqrt(np.asarray(bn2_v, np.float64) + EPS)
    bias4 = scale4 * np.asarray(bc2, np.float64) + (np.asarray(bn2_b, np.float64)
            - np.asarray(bn2_m, np.float64) * scale4)
    return dict(W1t=W1t, W2t=W2t, Wc1t=Wc1t, scale3=scale3, bias3=bias3,
                Wc2t=Wc2t, scale4=scale4, bias4=bias4)


def _np_dtype():
    return ml_dtypes.bfloat16 if DTYPE == "bf16" else np.float32


_BUILT = None


def build_bass():
    import concourse.bass as bass
    import concourse.mybir as mybir
    from concourse import tile
    from concourse.tile import add_dep_helper
    from bass_rust import AP

    dt = mybir.dt
    if DTYPE == "bf16":
        ddt, mdt = dt.bfloat16, dt.bfloat16
    elif DTYPE == "f32r":
        ddt, mdt = dt.float32, dt.float32r
    else:
        ddt, mdt = dt.float32, dt.float32

    nc = bass.Bass("TRN2", target_bir_lowering=False, debug=False,
                   num_devices=N_CORES)

    x_d = nc.dram_tensor("x", (NPC, 28, XW), ddt, kind="ExternalInput")
    w1s_d = nc.dram_tensor("w1s", (112, 2 * 144), ddt, kind="ExternalInput")
    w18_d = nc.dram_tensor("w18", (28, 144), ddt, kind="ExternalInput")
    w2a_d = nc.dram_tensor("w2ta", (128, 9 * 64), ddt, kind="ExternalInput")
    w2bs_d = nc.dram_tensor("w2bs", (128, 64), ddt, kind="ExternalInput")
    w2b8_d = nc.dram_tensor("w2b8", (17, 64), ddt, kind="ExternalInput")
    wc1s_d = nc.dram_tensor("wc1s", (128, 3 * 32), ddt, kind="ExternalInput")
    wc2s_d = nc.dram_tensor("wc2s", (96, 8), ddt, kind="ExternalInput")
    sb3_d = nc.dram_tensor("sb3", (32, 2), dt.float32, kind="ExternalInput")
    sb4_d = nc.dram_tensor("sb4", (8, 2), dt.float32, kind="ExternalInput")
    onesb_d = nc.dram_tensor("onesb", (17, XW), ddt, kind="ExternalInput")
    out_d = nc.dram_tensor("out", (NPC, 8, T), dt.float32, kind="ExternalOutput")

    LR = (mybir.ActivationFunctionType.Relu
          if os.environ.get("SIM_ACT") == "relu"
          else mybir.ActivationFunctionType.Lrelu)

    def mm(out, lhsT, rhs, start, stop):
        return nc.tensor.matmul(
            out, lhsT.bitcast(mdt) if mdt != ddt else lhsT,
            rhs.bitcast(mdt) if mdt != ddt else rhs,
            start=start, stop=stop)

    def make_ap(base, ap_list, extra_offset=0):
        return AP(tensor=base.tensor, offset=base.offset + extra_offset,
                  ap=ap_list, const_val=base.const_val,
                  runtime_checks=base.runtime_checks)

    with tile.TileContext(nc) as tc:
        with (
            tc.tile_pool(name="wpool", bufs=1) as wpool,
            tc.tile_pool(name="xpool", bufs=4) as xpool,
            tc.tile_pool(name="o1a", bufs=2) as o1ap,
            tc.tile_pool(name="o1b", bufs=2) as o1bp,
            tc.tile_pool(name="o2", bufs=2) as o2p,
            tc.tile_pool(name="o3", bufs=2) as o3p,
            tc.tile_pool(name="h2", bufs=2) as h2p,
            tc.tile_pool(name="bscr", bufs=2, space="DRAM") as bscrp,
            tc.tile_pool(name="ps1a", bufs=2, space="PSUM") as ps1ap,
            tc.tile_pool(name="ps1b", bufs=2, space="PSUM") as ps1bp,
            tc.tile_pool(name="ps2", bufs=2, space="PSUM") as ps2p,
            tc.tile_pool(name="ps3", bufs=1, space="PSUM") as ps3p,
            tc.tile_pool(name="ps4", bufs=1, space="PSUM") as ps4p,
        ):
            w1s = wpool.tile([112, 2 * 144], ddt)
            w18 = wpool.tile([28, 144], ddt)
            w2a = wpool.tile([128, 9 * 64], ddt)
            w2bs = wpool.tile([128, 64], ddt)
            w2b8 = wpool.tile([17, 64], ddt)
            wc1s = wpool.tile([128, 3 * 32], ddt)
            wc2s = wpool.tile([96, 8], ddt)
            sb3 = wpool.tile([32, 2], dt.float32)
            sb4 = wpool.tile([8, 2], dt.float32)
            zt = wpool.tile([128, 8], dt.float32)
            onesB = wpool.tile([17, XW], ddt)
            nc.gpsimd.memset(zt[:], 0.0)
            for tile_, dram in ((w1s, w1s_d), (w18, w18_d), (w2a, w2a_d),
                                (w2bs, w2bs_d), (w2b8, w2b8_d), (wc1s, wc1s_d),
                                (wc2s, wc2s_d), (sb3, sb3_d), (sb4, sb4_d),
                                (onesB, onesb_d)):
                nc.sync.dma_start(tile_[:], dram[:])

            for n in range(NPC):
                # --- stage-1 operand stacks: one windowed DMA each
                # xs1 rows 28g+r = x[n][r, g+j]   (taps 0-3)
                # xs2 rows 28g+r = x[n][r, 4+g+j] (taps 4-7)
                xs1 = xpool.tile([112, XW], ddt, tag="xs")
                xs2 = xpool.tile([112, XW], ddt, tag="xs")
                W1w = XW - 3
                W2w = XW - 7
                xn = x_d[n]
                nc.sync.dma_start(
                    xs1[:, 0:W1w],
                    make_ap(xn, [[1, 4], [XW, 28], [1, W1w]]))
                nc.sync.dma_start(
                    xs2[:, 0:W2w],
                    make_ap(xn, [[1, 4], [XW, 28], [1, W2w]], extra_offset=4))

                o1a = o1ap.tile([128, XW], ddt)
                o1b = o1bp.tile([17, XW], ddt)
                bstk = o1ap.tile([128, XW], ddt, tag="bstk")
                o2s = o2p.tile([128, O2W], ddt)
                o3s = o3p.tile([96, O3W], ddt)
                h2 = h2p.tile([8, T], dt.float32)

                # halo zeroing + ones row (same engine as the tile's writer)
                nc.vector.tensor_copy(o1a[:, 0:4], zt[:, 0:4])
                nc.vector.tensor_copy(o1a[:, T + 4:XW], zt[:, 0:8])
                # zeros rows 0-15 (halos) + ones row 16, in one aligned copy;
                # evictions overwrite the data region afterwards
                nc.vector.tensor_copy(o1b[:], onesB[:])
                nc.vector.tensor_copy(o2s[0:64, 0:2], zt[0:64, 0:2])
                nc.vector.tensor_copy(o2s[0:64, T + 2:O2W], zt[0:64, 0:4])
                nc.scalar.copy(o3s[0:32, 0:1], zt[0:32, 0:1])
                nc.scalar.copy(o3s[0:32, T + 1:O3W], zt[0:32, 0:3])

                # ---- stage 1: out1 (144ch) = 9-tap conv of x (28ch)
                for tt in range(2):
                    t0 = tt * 512
                    psA = ps1ap.tile([128, 512], dt.float32)
                    psB = ps1bp.tile([16, 512], dt.float32)
                    r1 = xs1[:, t0: t0 + 512]
                    r2 = xs2[:, t0: t0 + 512]
                    r8 = xs1[0:28, t0 + 8: t0 + 8 + 512]
                    mm(psA[:], w1s[:, 0:128], r1, True, False)
                    mm(psA[:], w1s[:, 144:272], r2, False, False)
                    mm(psA[:], w18[:, 0:128], r8, False, True)
                    mm(psB[:], w1s[:, 128:144], r1, True, False)
                    mm(psB[:], w1s[:, 272:288], r2, False, False)
                    mm(psB[:], w18[:, 128:144], r8, False, True)
                    nc.vector.tensor_copy(o1a[:, 4 + t0: 4 + t0 + 512], psA[:])
                    nc.vector.tensor_copy(o1b[0:16, 4 + t0: 4 + t0 + 512], psB[:])

                if STAGES < 2:
                    nc.vector.tensor_copy(h2[:, 0:T], o1a[0:8, 4:4 + T])
                    nc.sync.dma_start(out_d[n], h2[:])
                    continue

                # stacked B operand: bstk rows 16g+r = o1b[r, g+j] (taps 0-7),
                # one SBUF->SBUF DMA per tap block.
                for g in range(8):
                    nc.sync.dma_start(bstk[16 * g:16 * g + 16, 0:XW - g],
                                      o1b[0:16, g:XW])

                # ---- stage 2: out2 (48ch padded to 64) = 9-tap conv of out1
                for tt in range(2):
                    t0 = tt * 512
                    ps2 = ps2p.tile([64, 512], dt.float32)
                    for dtp in range(9):
                        mm(ps2[:], w2a[:, dtp * 64: (dtp + 1) * 64],
                           o1a[:, t0 + dtp: t0 + dtp + 512], dtp == 0, False)
                    mm(ps2[:], w2bs[:], bstk[:, t0: t0 + 512], False, False)
                    mm(ps2[:], w2b8[:], o1b[:, t0 + 8: t0 + 8 + 512], False, True)
                    nc.vector.tensor_copy(o2s[0:64, 2 + t0: 2 + t0 + 512], ps2[:])

                if STAGES < 3:
                    nc.vector.tensor_copy(h2[:, 0:T], o2s[0:8, 2:2 + T])
                    nc.sync.dma_start(out_d[n], h2[:])
                    continue

                # o2 replica shifted by one tap (rows 64-127, DVE part-shift)
                nc.vector.tensor_copy(o2s[64:128, 0:O2W - 1], o2s[0:64, 1:O2W])

                # ---- stage 3: h1 (16ch padded to 32) = 5-tap conv, bn+lrelu
                for tt in range(2):
                    t0 = tt * 512
                    ps3 = ps3p.tile([32, 512], dt.float32)
                    mm(ps3[:], wc1s[:, 0:32], o2s[:, t0: t0 + 512], True, False)
                    mm(ps3[:], wc1s[:, 32:64], o2s[:, t0 + 2: t0 + 2 + 512],
                       False, False)
                    mm(ps3[:], wc1s[0:64, 64:96], o2s[0:64, t0 + 4: t0 + 4 + 512],
                       False, True)
                    nc.scalar.activation(o3s[0:32, 1 + t0: 1 + t0 + 512], ps3[:],
                                         LR, bias=sb3[:, 1:2], scale=sb3[:, 0:1],
                                         alpha=0.01)

                if STAGES < 4:
                    nc.vector.tensor_copy(h2[:, 0:T], o3s[0:8, 1:1 + T])
                    nc.sync.dma_start(out_d[n], h2[:])
                    continue

                # h1 replicas shifted by 1 and 2 taps (DVE part-shift)
                nc.vector.tensor_copy(o3s[32:64, 0:O3W - 1], o3s[0:32, 1:O3W])
                nc.vector.tensor_copy(o3s[64:96, 0:O3W - 2], o3s[0:32, 2:O3W])

                # ---- stage 4: h2 (8ch) = 3-tap conv, bn+lrelu
                for tt in range(2):
                    t0 = tt * 512
                    ps4 = ps4p.tile([8, 512], dt.float32)
                    mm(ps4[:], wc2s[:], o3s[:, t0: t0 + 512], True, True)
                    nc.scalar.activation(h2[:, t0: t0 + 512], ps4[:], LR,
                                         bias=sb4[:, 1:2], scale=sb4[:, 0:1],
                                         alpha=0.01)

                nc.sync.dma_start(out_d[n], h2[:])

    # TRN2 engine instructions accept a single sync-wait command, but Tile's
    # wait assignment can emit several (fresh DMA tick + PSUM-WAR tick, ...).
    # Legalize in two steps:
    #  1. matmuls: move extras onto the paired LDWEIGHTS (runs strictly
    #     earlier on the PE FIFO, so the stall point only moves up);
    #  2. anything still over the cap: hoist extras onto standalone
    #     EventSemaphore instructions inserted just before, on the same
    #     engine (stalls the sequencer where the instruction would have
    #     stalled anyway).
    for b in nc.m.functions[0].blocks:
        insts = list(b.instructions)
        for k, inst in enumerate(insts):
            if type(inst).__name__ != "InstMatmult":
                continue
            si = inst.sync_info
            if not si or len(si.on_wait) <= 1:
                continue
            prev = insts[k - 1]
            if type(prev).__name__ != "InstLdweights":
                continue
            psi = prev.sync_info
            prev.sync_info = mybir.SyncInfo(
                on_wait=list(si.on_wait[1:]) + (list(psi.on_wait) if psi else []),
                on_update=(list(psi.on_update) if psi else []))
            inst.sync_info = mybir.SyncInfo(
                on_wait=[si.on_wait[0]], on_update=list(si.on_update))

    esc = 0
    for b in nc.m.functions[0].blocks:
        insts = list(b.instructions)
        out = []
        changed = False
        for inst in insts:
            si = inst.sync_info
            nw = len(si.on_wait) if si and si.on_wait else 0
            if nw > 1 and type(inst).__name__ != "InstEventSemaphore":
                waits = list(si.on_wait)
                for w in waits[:-1]:
                    esc += 1
                    es = mybir.InstEventSemaphore(
                        name=f"ES-legal-{esc}", engine=inst.engine,
                        ins=[], outs=[], bass_nofuse=True)
                    es.sync_info = mybir.SyncInfo(on_wait=[w], on_update=[])
                    out.append(es)
                inst.sync_info = mybir.SyncInfo(
                    on_wait=[waits[-1]], on_update=list(si.on_update))
                changed = True
            out.append(inst)
        if changed:
            b.instructions = out

    return nc


def host_prep(inputs):
    poses = np.asarray(inputs["poses"], np.float32)
    fw = fold_weights(**{k: np.asarray(v) for k, v in inputs.items()
                         if k != "poses"})
    npdt = _np_dtype()

    Xp = np.zeros((N, 28, XW), np.float32)
    Xp[:, :27, 4:4 + T] = poses.transpose(0, 2, 1)
    Xp[:, 27, :] = 1.0
    Xp = np.ascontiguousarray(Xp.astype(npdt))

    W1t, W2t, Wc1t, Wc2t = fw["W1t"], fw["W2t"], fw["Wc1t"], fw["Wc2t"]

    w1s = np.zeros((112, 2 * 144), np.float32)
    for g in range(4):
        w1s[28 * g:28 * g + 28, 0:144] = W1t[g]
        w1s[28 * g:28 * g + 28, 144:288] = W1t[4 + g]
    w18 = W1t[8].astype(np.float32)                  # (28, 144)

    w2ta = np.zeros((128, 9 * 64), np.float32)
    for dtp in range(9):
        w2ta[:, dtp * 64: dtp * 64 + 48] = W2t[dtp][:128]
    w2bs = np.zeros((128, 64), np.float32)
    for g in range(8):
        w2bs[16 * g:16 * g + 16, 0:48] = W2t[g][128:144]
    w2b8 = np.zeros((17, 64), np.float32)
    w2b8[:16, 0:48] = W2t[8][128:144]
    w2b8[16, 0:48] = W2t[4][144]        # bias row, applied once via ones row

    # s3 operand rows: 0-63 = out2(64pad) @ tap g, 64-127 = @ tap g+1
    wc1s = np.zeros((128, 3 * 32), np.float32)
    wc1s[0:48, 0:16] = Wc1t[0]; wc1s[64:112, 0:16] = Wc1t[1]
    wc1s[0:48, 32:48] = Wc1t[2]; wc1s[64:112, 32:48] = Wc1t[3]
    wc1s[0:48, 64:80] = Wc1t[4]

    # s4 operand rows: 0-31 = h1(32pad), 32-63 = h1@+1, 64-95 = h1@+2
    wc2s = np.zeros((96, 8), np.float32)
    wc2s[0:16] = Wc2t[0]
    wc2s[32:48] = Wc2t[1]
    wc2s[64:80] = Wc2t[2]

    sb3 = np.zeros((32, 2), np.float32)
    sb3[:16, 0] = fw["scale3"]; sb3[:16, 1] = fw["bias3"]
    sb3[16:, 0] = 1.0
    sb4 = np.stack([fw["scale4"], fw["bias4"]], axis=1).astype(np.float32)

    onesb = np.zeros((17, XW), np.float32)
    onesb[16, :] = 1.0

    common = dict(onesb=np.ascontiguousarray(onesb.astype(npdt)),
                  sb3=sb3, sb4=sb4,
                  w1s=np.ascontiguousarray(w1s.astype(npdt)),
                  w18=np.ascontiguousarray(w18.astype(npdt)),
                  w2ta=np.ascontiguousarray(w2ta.astype(npdt)),
                  w2bs=np.ascontiguousarray(w2bs.astype(npdt)),
                  w2b8=np.ascontiguousarray(w2b8.astype(npdt)),
                  wc1s=np.ascontiguousarray(wc1s.astype(npdt)),
                  wc2s=np.ascontiguousarray(wc2s.astype(npdt)))
    in_maps = []
    for c in range(N_CORES):
        m = dict(common)
        m["x"] = np.ascontiguousarray(Xp[c * NPC:(c + 1) * NPC])
        in_maps.append(m)
    return in_maps


def run(inputs, trace=False, tmpdir=None):
    global _BUILT
    from concourse import bass_utils
    if _BUILT is None:
        _BUILT = build_bass()
    nc = _BUILT
    in_maps = host_prep(inputs)
    res = bass_utils.run_bass_kernel_spmd(
        nc, in_maps, core_ids=list(range(N_CORES)), trace=trace,
        tmpdir=tmpdir)
    outs = [res.results[c]["out"] for c in range(N_CORES)]
    full = np.concatenate(outs, axis=0)          # (256, 8, 1024)
    return np.ascontiguousarray(full.transpose(0, 2, 1)).astype(np.float32), res


def kernel(**inputs) -> np.ndarray:
    out, _ = run(inputs, trace=False)
    return out


# revision 46
# speedup vs baseline: 1.0435x; 1.0435x over previous
"""AffEncoder Trainium2 kernel.

The network folds into 4 temporal-conv stages (channel-major):
  s1: K=28  (27 pose ch + ones row), M=144, 9 taps   (conv1 + A1 einsum folded)
  s2: K=145 (144 ch + ones row),     M=48,  9 taps   (gather + conv2 + A2 folded)
  s3: K=48, M=16, 5 taps, then Lrelu(scale*x+bias)   (convc1 + bn1 folded)
  s4: K=16, M=8,  3 taps, then Lrelu(scale*x+bias)   (convc2 + bn2 folded)

Sharding: pure data parallel, 32 batch elements per core across 8 cores.
Host does the (n,t,c)->(n,c,t) transposes + weight folding; the device runs
channel-major matmul pipelines.

Per batch element, per 512-col t-tile:
  s1: taps 0-3 / 4-7 pre-shifted into two 112-row operand stacks (one
      windowed DMA each); tap 8 = stack1 block 0 at window +8     -> 6 MM
  s2: 9 full-K passes (ch 0-127) + stacked B pass (ch 128-143 x 8 taps,
      one windowed SBUF->SBUF DMA) + tap-8/bias pass              -> 11 MM
  s3: K=128 double-tap passes on a partition-shifted replica      -> 3 MM
  s4: split K=32 (ACT-written rows) + K=64 (DVE replica rows)     -> 2 MM

TRN2 matmuls accept only ONE sync-wait, so each matmul's operands live in
tiles with a single producer: xs1/xs2/bstk are each written by exactly one
DMA instruction; o1a/o1b/o2s and the o3s replica rows are written only by
DVE; o3s rows 0-31 only by ACT.  Channels are padded (48->64, 16->32) so
partition-shifted replicas start 32-aligned.
"""
import os
import sys
import numpy as np

for _p in ("/opt/trn_rl_repo",):
    if _p not in sys.path and os.path.isdir(_p):
        sys.path.insert(0, _p)

import ml_dtypes  # noqa: E402

N_CORES = 8
N, T = 256, 1024
NPC = N // N_CORES
EPS = 1e-5
J, C, K1, K2, F1, F2 = 9, 3, 5, 3, 16, 16
NUM_PARTS, MAX_EDGES = 3, 3

XW = T + 12                 # x pad: 4 left, 8 right
O2W = T + 6                 # o2s pad: 2 left, 4 right
O3W = T + 4                 # o3s pad: 1 left, 3 right
DTYPE = os.environ.get("BASS_DTYPE", "bf16")  # bf16 | f32r | f32
STAGES = int(os.environ.get("STAGES", "4"))   # debug: truncate pipeline


def fold_weights(W1, b1, A1, W2, b2, A2, Wc1, bc1, bn1_w, bn1_b, bn1_m, bn1_v,
                 Wc2, bc2, bn2_w, bn2_b, bn2_m, bn2_v):
    W1 = np.asarray(W1, np.float64); A1 = np.asarray(A1, np.float64)
    W2 = np.asarray(W2, np.float64); A2 = np.asarray(A2, np.float64)

    W1r = W1[:, :, :, 0].reshape(K1, F1, C, 9)              # [k, c, ci, dt]
    W1t = np.zeros((9, 28, 144))
    W1t[:, :27, :] = np.einsum('kcid,kvw->dvicw', W1r, A1).reshape(9, 27, 144)
    beff1 = np.einsum('kc,kw->cw', np.asarray(b1, np.float64).reshape(K1, F1),
                      A1.sum(axis=1)).reshape(144)
    W1t[4, 27, :] = beff1

    W2r = W2[:, :, :, 0].reshape(K2, F2, F1, MAX_EDGES, 9)  # [k2, c2, c, e, dt]
    W2t = np.zeros((9, 145, 48))
    W2t[:, :144, :] = np.einsum('kbced,kpq->dcpebq', W2r, A2).reshape(9, 144, 48)
    beff2 = np.einsum('kb,kq->bq', np.asarray(b2, np.float64).reshape(K2, F2),
                      A2.sum(axis=1)).reshape(48)
    W2t[4, 144, :] = beff2

    Wc1t = np.asarray(Wc1, np.float64).transpose(2, 1, 0)   # [dt, m2, c3]
    scale3 = np.asarray(bn1_w, np.float64) / np.sqrt(np.asarray(bn1_v, np.float64) + EPS)
    bias3 = scale3 * np.asarray(bc1, np.float64) + (np.asarray(bn1_b, np.float64)
            - np.asarray(bn1_m, np.float64) * scale3)
    Wc2t = np.asarray(Wc2, np.float64).transpose(2, 1, 0)   # [dt, c3, c4]
    scale4 = np.asarray(bn2_w, np.float64) / np.sqrt(np.asarray(bn2_v, np.float64) + EPS)
    bias4 = scale4 * np.asarray(bc2, np.float64) + (np.asarray(bn2_b, np.float64)
            - np.asarray(bn2_m, np.float64) * scale4)
    return dict(W1t=W1t, W2t=W2t, Wc1t=Wc1t, scale3=scale3, bias3=bias3,
                Wc2t=Wc2t, scale4=scale4, bias4=bias4)


def _np_dtype():
    return ml_dtypes.bfloat16 if DTYPE == "bf16" else np.float32


_BUILT = None


def build_bass():
    import concourse.bass as bass
    import concourse.mybir as mybir
    from concourse import tile
    from concourse.tile import add_dep_helper
    from bass_rust import AP

    dt = mybir.dt
    if DTYPE == "bf16":
        ddt, mdt = dt.bfloat16, dt.bfloat16
    elif DTYPE == "f32r":
        ddt, mdt = dt.float32, dt.float32r
    else:
        ddt, mdt = dt.float32, dt.float32

    nc = bass.Bass("TRN2", target_bir_lowering=False, debug=False,
                   num_devices=N_CORES)

    x_d = nc.dram_tensor("x", (NPC, 28, XW), ddt, kind="ExternalInput")
    w1s_d = nc.dram_tensor("w1s", (126, 2 * 144), ddt, kind="ExternalInput")
    w2a_d = nc.dram_tensor("w2ta", (128, 9 * 64), ddt, kind="ExternalInput")
    w2bs_d = nc.dram_tensor("w2bs", (128, 64), ddt, kind="ExternalInput")
    w2b8_d = nc.dram_tensor("w2b8", (17, 64), ddt, kind="ExternalInput")
    wc1s_d = nc.dram_tensor("wc1s", (128, 3 * 32), ddt, kind="ExternalInput")
    wc2s_d = nc.dram_tensor("wc2s", (96, 8), ddt, kind="ExternalInput")
    sb3_d = nc.dram_tensor("sb3", (32, 2), dt.float32, kind="ExternalInput")
    sb4_d = nc.dram_tensor("sb4", (8, 2), dt.float32, kind="ExternalInput")
    onesb_d = nc.dram_tensor("onesb", (17, XW), ddt, kind="ExternalInput")
    out_d = nc.dram_tensor("out", (NPC, 8, T), dt.float32, kind="ExternalOutput")

    LR = (mybir.ActivationFunctionType.Relu
          if os.environ.get("SIM_ACT") == "relu"
          else mybir.ActivationFunctionType.Lrelu)

    def mm(out, lhsT, rhs, start, stop):
        return nc.tensor.matmul(
            out, lhsT.bitcast(mdt) if mdt != ddt else lhsT,
            rhs.bitcast(mdt) if mdt != ddt else rhs,
            start=start, stop=stop)

    def make_ap(base, ap_list, extra_offset=0):
        return AP(tensor=base.tensor, offset=base.offset + extra_offset,
                  ap=ap_list, const_val=base.const_val,
                  runtime_checks=base.runtime_checks)

    with tile.TileContext(nc) as tc:
        with (
            tc.tile_pool(name="wpool", bufs=1) as wpool,
            tc.tile_pool(name="xpool", bufs=4) as xpool,
            tc.tile_pool(name="o1a", bufs=2) as o1ap,
            tc.tile_pool(name="o1b", bufs=2) as o1bp,
            tc.tile_pool(name="o2", bufs=2) as o2p,
            tc.tile_pool(name="o3", bufs=2) as o3p,
            tc.tile_pool(name="h2", bufs=2) as h2p,
            tc.tile_pool(name="bscr", bufs=2, space="DRAM") as bscrp,
            tc.tile_pool(name="ps1a", bufs=2, space="PSUM") as ps1ap,
            tc.tile_pool(name="ps1b", bufs=2, space="PSUM") as ps1bp,
            tc.tile_pool(name="ps2", bufs=2, space="PSUM") as ps2p,
            tc.tile_pool(name="ps3", bufs=1, space="PSUM") as ps3p,
            tc.tile_pool(name="ps4", bufs=1, space="PSUM") as ps4p,
        ):
            w1s = wpool.tile([126, 2 * 144], ddt)
            w2a = wpool.tile([128, 9 * 64], ddt)
            w2bs = wpool.tile([128, 64], ddt)
            w2b8 = wpool.tile([17, 64], ddt)
            wc1s = wpool.tile([128, 3 * 32], ddt)
            wc2s = wpool.tile([96, 8], ddt)
            sb3 = wpool.tile([32, 2], dt.float32)
            sb4 = wpool.tile([8, 2], dt.float32)
            zt = wpool.tile([128, 8], dt.float32)
            onesB = wpool.tile([17, XW], ddt)
            nc.gpsimd.memset(zt[:], 0.0)
            for tile_, dram in ((w1s, w1s_d), (w2a, w2a_d),
                                (w2bs, w2bs_d), (w2b8, w2b8_d), (wc1s, wc1s_d),
                                (wc2s, wc2s_d), (sb3, sb3_d), (sb4, sb4_d),
                                (onesB, onesb_d)):
                nc.sync.dma_start(tile_[:], dram[:])

            for n in range(NPC):
                # --- stage-1 operand stacks: windowed DMA for the 4 tap
                # blocks + a half of tap 8 in rows 112-125.
                # xs1 rows 28g+r = x[n][r, g+j]   (taps 0-3), rows 112-125 =
                #     x[n][0:14, 8+j] (tap 8, channels 0-13)
                # xs2 rows 28g+r = x[n][r, 4+g+j] (taps 4-7), rows 112-125 =
                #     x[n][14:28, 8+j] (tap 8, channels 14-27)
                xs1 = xpool.tile([126, XW], ddt, tag="xs")
                xs2 = xpool.tile([126, XW], ddt, tag="xs")
                W1w = XW - 3
                W2w = XW - 7
                W8w = XW - 8
                xn = x_d[n]
                nc.sync.dma_start(
                    xs1[0:112, 0:W1w],
                    make_ap(xn, [[1, 4], [XW, 28], [1, W1w]]))
                nc.sync.dma_start(
                    xs2[0:112, 0:W2w],
                    make_ap(xn, [[1, 4], [XW, 28], [1, W2w]], extra_offset=4))
                nc.sync.dma_start(xs1[112:126, 0:W8w], xn[0:14, 8:XW])
                nc.sync.dma_start(xs2[112:126, 0:W8w], xn[14:28, 8:XW])

                o1a = o1ap.tile([128, XW], ddt)
                o1b = o1bp.tile([17, XW], ddt)
                bstk = o1ap.tile([128, XW], ddt, tag="bstk")
                o2s = o2p.tile([128, O2W], ddt)
                o3s = o3p.tile([96, O3W], ddt)
                h2 = h2p.tile([8, T], dt.float32)

                # halo zeroing + ones row (same engine as the tile's writer)
                nc.vector.tensor_copy(o1a[:, 0:4], zt[:, 0:4])
                nc.vector.tensor_copy(o1a[:, T + 4:XW], zt[:, 0:8])
                # zeros rows 0-15 (halos) + ones row 16, in one aligned copy;
                # evictions overwrite the data region afterwards
                nc.vector.tensor_copy(o1b[:], onesB[:])
                nc.vector.tensor_copy(o2s[0:64, 0:2], zt[0:64, 0:2])
                nc.vector.tensor_copy(o2s[0:64, T + 2:O2W], zt[0:64, 0:4])
                nc.scalar.copy(o3s[0:32, 0:1], zt[0:32, 0:1])
                nc.scalar.copy(o3s[0:32, T + 1:O3W], zt[0:32, 0:3])

                # ---- stage 1: out1 (144ch) = 9-tap conv of x (28ch)
                for tt in range(2):
                    t0 = tt * 512
                    psA = ps1ap.tile([128, 512], dt.float32)
                    psB = ps1bp.tile([16, 512], dt.float32)
                    r1 = xs1[:, t0: t0 + 512]
                    r2 = xs2[:, t0: t0 + 512]
                    mm(psA[:], w1s[:, 0:128], r1, True, False)
                    mm(psA[:], w1s[:, 144:272], r2, False, True)
                    mm(psB[:], w1s[:, 128:144], r1, True, False)
                    mm(psB[:], w1s[:, 272:288], r2, False, True)
                    nc.vector.tensor_copy(o1a[:, 4 + t0: 4 + t0 + 512], psA[:])
                    nc.vector.tensor_copy(o1b[0:16, 4 + t0: 4 + t0 + 512], psB[:])

                if STAGES < 2:
                    nc.vector.tensor_copy(h2[:, 0:T], o1a[0:8, 4:4 + T])
                    nc.sync.dma_start(out_d[n], h2[:])
                    continue

                # stacked B operand: bstk rows 16g+r = o1b[r, g+j] (taps 0-7),
                # one SBUF->SBUF DMA per tap block.
                for g in range(8):
                    nc.sync.dma_start(bstk[16 * g:16 * g + 16, 0:XW - g],
                                      o1b[0:16, g:XW])

                # ---- stage 2: out2 (48ch padded to 64) = 9-tap conv of out1
                for tt in range(2):
                    t0 = tt * 512
                    ps2 = ps2p.tile([64, 512], dt.float32)
                    for dtp in range(9):
                        mm(ps2[:], w2a[:, dtp * 64: (dtp + 1) * 64],
                           o1a[:, t0 + dtp: t0 + dtp + 512], dtp == 0, False)
                    mm(ps2[:], w2bs[:], bstk[:, t0: t0 + 512], False, False)
                    mm(ps2[:], w2b8[:], o1b[:, t0 + 8: t0 + 8 + 512], False, True)
                    nc.vector.tensor_copy(o2s[0:64, 2 + t0: 2 + t0 + 512], ps2[:])

                if STAGES < 3:
                    nc.vector.tensor_copy(h2[:, 0:T], o2s[0:8, 2:2 + T])
                    nc.sync.dma_start(out_d[n], h2[:])
                    continue

                # o2 replica shifted by one tap (rows 64-127, DVE part-shift)
                nc.vector.tensor_copy(o2s[64:128, 0:O2W - 1], o2s[0:64, 1:O2W])

                # ---- stage 3: h1 (16ch padded to 32) = 5-tap conv, bn+lrelu
                for tt in range(2):
                    t0 = tt * 512
                    ps3 = ps3p.tile([32, 512], dt.float32)
                    mm(ps3[:], wc1s[:, 0:32], o2s[:, t0: t0 + 512], True, False)
                    mm(ps3[:], wc1s[:, 32:64], o2s[:, t0 + 2: t0 + 2 + 512],
                       False, False)
                    mm(ps3[:], wc1s[0:64, 64:96], o2s[0:64, t0 + 4: t0 + 4 + 512],
                       False, True)
                    nc.scalar.activation(o3s[0:32, 1 + t0: 1 + t0 + 512], ps3[:],
                                         LR, bias=sb3[:, 1:2], scale=sb3[:, 0:1],
                                         alpha=0.01)

                if STAGES < 4:
                    nc.vector.tensor_copy(h2[:, 0:T], o3s[0:8, 1:1 + T])
                    nc.sync.dma_start(out_d[n], h2[:])
                    continue

                # h1 replicas shifted by 1 and 2 taps (DVE part-shift)
                nc.vector.tensor_copy(o3s[32:64, 0:O3W - 1], o3s[0:32, 1:O3W])
                nc.vector.tensor_copy(o3s[64:96, 0:O3W - 2], o3s[0:32, 2:O3W])

                # ---- stage 4: h2 (8ch) = 3-tap conv, bn+lrelu
                for tt in range(2):
                    t0 = tt * 512
                    ps4 = ps4p.tile([8, 512], dt.float32)
                    mm(ps4[:], wc2s[:], o3s[:, t0: t0 + 512], True, True)
                    nc.scalar.activation(h2[:, t0: t0 + 512], ps4[:], LR,
                                         bias=sb4[:, 1:2], scale=sb4[:, 0:1],
                                         alpha=0.01)

                nc.sync.dma_start(out_d[n], h2[:])

    # TRN2 engine instructions accept a single sync-wait command, but Tile's
    # wait assignment can emit several (fresh DMA tick + PSUM-WAR tick, ...).
    # Legalize in two steps:
    #  1. matmuls: move extras onto the paired LDWEIGHTS (runs strictly
    #     earlier on the PE FIFO, so the stall point only moves up);
    #  2. anything still over the cap: hoist extras onto standalone
    #     EventSemaphore instructions inserted just before, on the same
    #     engine (stalls the sequencer where the instruction would have
    #     stalled anyway).
    for b in nc.m.functions[0].blocks:
        insts = list(b.instructions)
        for k, inst in enumerate(insts):
            if type(inst).__name__ != "InstMatmult":
                continue
            si = inst.sync_info
            if not si or len(si.on_wait) <= 1:
                continue
            prev = insts[k - 1]
            if type(prev).__name__ != "InstLdweights":
                continue
            psi = prev.sync_info
            prev.sync_info = mybir.SyncInfo(
                on_wait=list(si.on_wait[1:]) + (list(psi.on_wait) if psi else []),
                on_update=(list(psi.on_update) if psi else []))
            inst.sync_info = mybir.SyncInfo(
                on_wait=[si.on_wait[0]], on_update=list(si.on_update))

    esc = 0
    for b in nc.m.functions[0].blocks:
        insts = list(b.instructions)
        out = []
        changed = False
        for inst in insts:
            si = inst.sync_info
            nw = len(si.on_wait) if si and si.on_wait else 0
            if nw > 1 and type(inst).__name__ != "InstEventSemaphore":
                waits = list(si.on_wait)
                for w in waits[:-1]:
                    esc += 1
                    es = mybir.InstEventSemaphore(
                        name=f"ES-legal-{esc}", engine=inst.engine,
                        ins=[], outs=[], bass_nofuse=True)
                    es.sync_info = mybir.SyncInfo(on_wait=[w], on_update=[])
                    out.append(es)
                inst.sync_info = mybir.SyncInfo(
                    on_wait=[waits[-1]], on_update=list(si.on_update))
                changed = True
            out.append(inst)
        if changed:
            b.instructions = out

    return nc


def host_prep(inputs):
    poses = np.asarray(inputs["poses"], np.float32)
    fw = fold_weights(**{k: np.asarray(v) for k, v in inputs.items()
                         if k != "poses"})
    npdt = _np_dtype()

    Xp = np.zeros((N, 28, XW), np.float32)
    Xp[:, :27, 4:4 + T] = poses.transpose(0, 2, 1)
    Xp[:, 27, :] = 1.0
    Xp = np.ascontiguousarray(Xp.astype(npdt))

    W1t, W2t, Wc1t, Wc2t = fw["W1t"], fw["W2t"], fw["Wc1t"], fw["Wc2t"]

    w1s = np.zeros((126, 2 * 144), np.float32)
    for g in range(4):
        w1s[28 * g:28 * g + 28, 0:144] = W1t[g]
        w1s[28 * g:28 * g + 28, 144:288] = W1t[4 + g]
    w1s[112:126, 0:144] = W1t[8][0:14]
    w1s[112:126, 144:288] = W1t[8][14:28]

    w2ta = np.zeros((128, 9 * 64), np.float32)
    for dtp in range(9):
        w2ta[:, dtp * 64: dtp * 64 + 48] = W2t[dtp][:128]
    w2bs = np.zeros((128, 64), np.float32)
    for g in range(8):
        w2bs[16 * g:16 * g + 16, 0:48] = W2t[g][128:144]
    w2b8 = np.zeros((17, 64), np.float32)
    w2b8[:16, 0:48] = W2t[8][128:144]
    w2b8[16, 0:48] = W2t[4][144]        # bias row, applied once via ones row

    # s3 operand rows: 0-63 = out2(64pad) @ tap g, 64-127 = @ tap g+1
    wc1s = np.zeros((128, 3 * 32), np.float32)
    wc1s[0:48, 0:16] = Wc1t[0]; wc1s[64:112, 0:16] = Wc1t[1]
    wc1s[0:48, 32:48] = Wc1t[2]; wc1s[64:112, 32:48] = Wc1t[3]
    wc1s[0:48, 64:80] = Wc1t[4]

    # s4 operand rows: 0-31 = h1(32pad), 32-63 = h1@+1, 64-95 = h1@+2
    wc2s = np.zeros((96, 8), np.float32)
    wc2s[0:16] = Wc2t[0]
    wc2s[32:48] = Wc2t[1]
    wc2s[64:80] = Wc2t[2]

    sb3 = np.zeros((32, 2), np.float32)
    sb3[:16, 0] = fw["scale3"]; sb3[:16, 1] = fw["bias3"]
    sb3[16:, 0] = 1.0
    sb4 = np.stack([fw["scale4"], fw["bias4"]], axis=1).astype(np.float32)

    onesb = np.zeros((17, XW), np.float32)
    onesb[16, :] = 1.0

    common = dict(onesb=np.ascontiguousarray(onesb.astype(npdt)),
                  sb3=sb3, sb4=sb4,
                  w1s=np.ascontiguousarray(w1s.astype(npdt)),
                  w2ta=np.ascontiguousarray(w2ta.astype(npdt)),
                  w2bs=np.ascontiguousarray(w2bs.astype(npdt)),
                  w2b8=np.ascontiguousarray(w2b8.astype(npdt)),
                  wc1s=np.ascontiguousarray(wc1s.astype(npdt)),
                  wc2s=np.ascontiguousarray(wc2s.astype(npdt)))
    in_maps = []
    for c in range(N_CORES):
        m = dict(common)
        m["x"] = np.ascontiguousarray(Xp[c * NPC:(c + 1) * NPC])
        in_maps.append(m)
    return in_maps


def run(inputs, trace=False, tmpdir=None):
    global _BUILT
    from concourse import bass_utils
    if _BUILT is None:
        _BUILT = build_bass()
    nc = _BUILT
    in_maps = host_prep(inputs)
    res = bass_utils.run_bass_kernel_spmd(
        nc, in_maps, core_ids=list(range(N_CORES)), trace=trace,
        tmpdir=tmpdir)
    outs = [res.results[c]["out"] for c in range(N_CORES)]
    full = np.concatenate(outs, axis=0)          # (256, 8, 1024)
    return np.ascontiguousarray(full.transpose(0, 2, 1)).astype(np.float32), res


def kernel(**inputs) -> np.ndarray:
    out, _ = run(inputs, trace=False)
    return out


# revision 47
# speedup vs baseline: 1.1858x; 1.1364x over previous
"""AffEncoder Trainium2 kernel.

The network folds into 4 temporal-conv stages (channel-major):
  s1: K=28  (27 pose ch + ones row), M=144, 9 taps   (conv1 + A1 einsum folded)
  s2: K=145 (144 ch + ones row),     M=48,  9 taps   (gather + conv2 + A2 folded)
  s3: K=48, M=16, 5 taps, then Lrelu(scale*x+bias)   (convc1 + bn1 folded)
  s4: K=16, M=8,  3 taps, then Lrelu(scale*x+bias)   (convc2 + bn2 folded)

Sharding: pure data parallel, 32 batch elements per core across 8 cores.
Host does the (n,t,c)->(n,c,t) transposes + weight folding; the device runs
channel-major matmul pipelines.

Per batch element, per 512-col t-tile:
  s1: taps 0-3 / 4-7 pre-shifted into two 112-row operand stacks (one
      windowed DMA each); tap 8 = stack1 block 0 at window +8     -> 6 MM
  s2: 9 full-K passes (ch 0-127) + stacked B pass (ch 128-143 x 8 taps,
      one windowed SBUF->SBUF DMA) + tap-8/bias pass              -> 11 MM
  s3: K=128 double-tap passes on a partition-shifted replica      -> 3 MM
  s4: split K=32 (ACT-written rows) + K=64 (DVE replica rows)     -> 2 MM

TRN2 matmuls accept only ONE sync-wait, so each matmul's operands live in
tiles with a single producer: xs1/xs2/bstk are each written by exactly one
DMA instruction; o1a/o1b/o2s and the o3s replica rows are written only by
DVE; o3s rows 0-31 only by ACT.  Channels are padded (48->64, 16->32) so
partition-shifted replicas start 32-aligned.
"""
import os
import sys
import numpy as np

for _p in ("/opt/trn_rl_repo",):
    if _p not in sys.path and os.path.isdir(_p):
        sys.path.insert(0, _p)

import ml_dtypes  # noqa: E402

N_CORES = 8
N, T = 256, 1024
NPC = N // N_CORES
EPS = 1e-5
J, C, K1, K2, F1, F2 = 9, 3, 5, 3, 16, 16
NUM_PARTS, MAX_EDGES = 3, 3

XW = T + 12                 # x pad: 4 left, 8 right
O2W = T + 6                 # o2s pad: 2 left, 4 right
O3W = T + 4                 # o3s pad: 1 left, 3 right
DTYPE = os.environ.get("BASS_DTYPE", "bf16")  # bf16 | f32r | f32
STAGES = int(os.environ.get("STAGES", "4"))   # debug: truncate pipeline


def fold_weights(W1, b1, A1, W2, b2, A2, Wc1, bc1, bn1_w, bn1_b, bn1_m, bn1_v,
                 Wc2, bc2, bn2_w, bn2_b, bn2_m, bn2_v):
    W1 = np.asarray(W1, np.float64); A1 = np.asarray(A1, np.float64)
    W2 = np.asarray(W2, np.float64); A2 = np.asarray(A2, np.float64)

    W1r = W1[:, :, :, 0].reshape(K1, F1, C, 9)              # [k, c, ci, dt]
    W1t = np.zeros((9, 28, 144))
    W1t[:, :27, :] = np.einsum('kcid,kvw->dvicw', W1r, A1).reshape(9, 27, 144)
    beff1 = np.einsum('kc,kw->cw', np.asarray(b1, np.float64).reshape(K1, F1),
                      A1.sum(axis=1)).reshape(144)
    W1t[4, 27, :] = beff1

    W2r = W2[:, :, :, 0].reshape(K2, F2, F1, MAX_EDGES, 9)  # [k2, c2, c, e, dt]
    W2t = np.zeros((9, 145, 48))
    W2t[:, :144, :] = np.einsum('kbced,kpq->dcpebq', W2r, A2).reshape(9, 144, 48)
    beff2 = np.einsum('kb,kq->bq', np.asarray(b2, np.float64).reshape(K2, F2),
                      A2.sum(axis=1)).reshape(48)
    W2t[4, 144, :] = beff2

    Wc1t = np.asarray(Wc1, np.float64).transpose(2, 1, 0)   # [dt, m2, c3]
    scale3 = np.asarray(bn1_w, np.float64) / np.sqrt(np.asarray(bn1_v, np.float64) + EPS)
    bias3 = scale3 * np.asarray(bc1, np.float64) + (np.asarray(bn1_b, np.float64)
            - np.asarray(bn1_m, np.float64) * scale3)
    Wc2t = np.asarray(Wc2, np.float64).transpose(2, 1, 0)   # [dt, c3, c4]
    scale4 = np.asarray(bn2_w, np.float64) / np.sqrt(np.asarray(bn2_v, np.float64) + EPS)
    bias4 = scale4 * np.asarray(bc2, np.float64) + (np.asarray(bn2_b, np.float64)
            - np.asarray(bn2_m, np.float64) * scale4)
    return dict(W1t=W1t, W2t=W2t, Wc1t=Wc1t, scale3=scale3, bias3=bias3,
                Wc2t=Wc2t, scale4=scale4, bias4=bias4)


def _np_dtype():
    return ml_dtypes.bfloat16 if DTYPE == "bf16" else np.float32


_BUILT = None


def build_bass():
    import concourse.bass as bass
    import concourse.mybir as mybir
    from concourse import tile
    from concourse.tile import add_dep_helper
    from bass_rust import AP

    dt = mybir.dt
    if DTYPE == "bf16":
        ddt, mdt = dt.bfloat16, dt.bfloat16
    elif DTYPE == "f32r":
        ddt, mdt = dt.float32, dt.float32r
    else:
        ddt, mdt = dt.float32, dt.float32

    nc = bass.Bass("TRN2", target_bir_lowering=False, debug=False,
                   num_devices=N_CORES)

    x_d = nc.dram_tensor("x", (NPC, 28, XW), ddt, kind="ExternalInput")
    w1s_d = nc.dram_tensor("w1s", (126, 2 * 144), ddt, kind="ExternalInput")
    w2a_d = nc.dram_tensor("w2ta", (128, 9 * 64), ddt, kind="ExternalInput")
    w2bs_d = nc.dram_tensor("w2bs", (128, 64), ddt, kind="ExternalInput")
    w2b8_d = nc.dram_tensor("w2b8", (17, 64), ddt, kind="ExternalInput")
    wc1s_d = nc.dram_tensor("wc1s", (128, 3 * 32), ddt, kind="ExternalInput")
    wc2s_d = nc.dram_tensor("wc2s", (96, 8), ddt, kind="ExternalInput")
    sb3_d = nc.dram_tensor("sb3", (32, 2), dt.float32, kind="ExternalInput")
    sb4_d = nc.dram_tensor("sb4", (8, 2), dt.float32, kind="ExternalInput")
    onesb_d = nc.dram_tensor("onesb", (17, XW), ddt, kind="ExternalInput")
    out_d = nc.dram_tensor("out", (NPC, 8, T), dt.float32, kind="ExternalOutput")

    LR = (mybir.ActivationFunctionType.Relu
          if os.environ.get("SIM_ACT") == "relu"
          else mybir.ActivationFunctionType.Lrelu)

    def mm(out, lhsT, rhs, start, stop):
        return nc.tensor.matmul(
            out, lhsT.bitcast(mdt) if mdt != ddt else lhsT,
            rhs.bitcast(mdt) if mdt != ddt else rhs,
            start=start, stop=stop)

    def make_ap(base, ap_list, extra_offset=0):
        return AP(tensor=base.tensor, offset=base.offset + extra_offset,
                  ap=ap_list, const_val=base.const_val,
                  runtime_checks=base.runtime_checks)

    with tile.TileContext(nc) as tc:
        with (
            tc.tile_pool(name="wpool", bufs=1) as wpool,
            tc.tile_pool(name="xpool", bufs=4) as xpool,
            tc.tile_pool(name="o1a", bufs=2) as o1ap,
            tc.tile_pool(name="o1b", bufs=2) as o1bp,
            tc.tile_pool(name="o2", bufs=2) as o2p,
            tc.tile_pool(name="o3", bufs=2) as o3p,
            tc.tile_pool(name="h2", bufs=2) as h2p,
            tc.tile_pool(name="bscr", bufs=2, space="DRAM") as bscrp,
            tc.tile_pool(name="ps1a", bufs=2, space="PSUM") as ps1ap,
            tc.tile_pool(name="ps1b", bufs=2, space="PSUM") as ps1bp,
            tc.tile_pool(name="ps2", bufs=2, space="PSUM") as ps2p,
            tc.tile_pool(name="ps3", bufs=1, space="PSUM") as ps3p,
            tc.tile_pool(name="ps4", bufs=1, space="PSUM") as ps4p,
        ):
            w1s = wpool.tile([126, 2 * 144], ddt)
            w2a = wpool.tile([128, 9 * 64], ddt)
            w2bs = wpool.tile([128, 64], ddt)
            w2b8 = wpool.tile([17, 64], ddt)
            wc1s = wpool.tile([128, 3 * 32], ddt)
            wc2s = wpool.tile([96, 8], ddt)
            sb3 = wpool.tile([32, 2], dt.float32)
            sb4 = wpool.tile([8, 2], dt.float32)
            zt = wpool.tile([128, 8], dt.float32)
            onesB = wpool.tile([17, XW], ddt)
            nc.gpsimd.memset(zt[:], 0.0)
            for tile_, dram in ((w1s, w1s_d), (w2a, w2a_d),
                                (w2bs, w2bs_d), (w2b8, w2b8_d), (wc1s, wc1s_d),
                                (wc2s, wc2s_d), (sb3, sb3_d), (sb4, sb4_d),
                                (onesB, onesb_d)):
                nc.sync.dma_start(tile_[:], dram[:])

            for n in range(NPC):
                # --- stage-1 operand stacks: windowed DMA for the 4 tap
                # blocks + a half of tap 8 in rows 112-125.
                # xs1 rows 28g+r = x[n][r, g+j]   (taps 0-3), rows 112-125 =
                #     x[n][0:14, 8+j] (tap 8, channels 0-13)
                # xs2 rows 28g+r = x[n][r, 4+g+j] (taps 4-7), rows 112-125 =
                #     x[n][14:28, 8+j] (tap 8, channels 14-27)
                xs1 = xpool.tile([126, XW], ddt, tag="xs")
                xs2 = xpool.tile([126, XW], ddt, tag="xs")
                W1w = XW - 3
                W2w = XW - 7
                W8w = XW - 8
                xn = x_d[n]
                nc.sync.dma_start(
                    xs1[0:112, 0:W1w],
                    make_ap(xn, [[1, 4], [XW, 28], [1, W1w]]))
                nc.sync.dma_start(
                    xs2[0:112, 0:W2w],
                    make_ap(xn, [[1, 4], [XW, 28], [1, W2w]], extra_offset=4))
                nc.sync.dma_start(xs1[112:126, 0:W8w], xn[0:14, 8:XW])
                nc.sync.dma_start(xs2[112:126, 0:W8w], xn[14:28, 8:XW])

                o1a = o1ap.tile([128, XW], ddt)
                o1b = o1bp.tile([17, XW], ddt)
                bstk = o1ap.tile([128, XW], ddt, tag="bstk")
                o2s = o2p.tile([128, O2W], ddt)
                o3s = o3p.tile([96, O3W], ddt)
                h2 = h2p.tile([8, T], dt.float32)

                # halo zeroing + ones row (same engine as the tile's writer)
                nc.vector.tensor_copy(o1a[:, 0:4], zt[:, 0:4])
                nc.vector.tensor_copy(o1a[:, T + 4:XW], zt[:, 0:8])
                # zeros rows 0-15 (halos) + ones row 16, in one aligned copy;
                # evictions overwrite the data region afterwards
                nc.vector.tensor_copy(o1b[:], onesB[:])
                nc.vector.tensor_copy(o2s[0:64, 0:2], zt[0:64, 0:2])
                nc.vector.tensor_copy(o2s[0:64, T + 2:O2W], zt[0:64, 0:4])
                nc.scalar.copy(o3s[0:32, 0:1], zt[0:32, 0:1])
                nc.scalar.copy(o3s[0:32, T + 1:O3W], zt[0:32, 0:3])

                # ---- stage 1: out1 (144ch) = 9-tap conv of x (28ch)
                for tt in range(2):
                    t0 = tt * 512
                    psA = ps1ap.tile([128, 512], dt.float32)
                    psB = ps1bp.tile([16, 512], dt.float32)
                    r1 = xs1[:, t0: t0 + 512]
                    r2 = xs2[:, t0: t0 + 512]
                    mm(psA[:], w1s[:, 0:128], r1, True, False)
                    mm(psA[:], w1s[:, 144:272], r2, False, True)
                    mm(psB[:], w1s[:, 128:144], r1, True, False)
                    mm(psB[:], w1s[:, 272:288], r2, False, True)
                    nc.vector.tensor_copy(o1a[:, 4 + t0: 4 + t0 + 512], psA[:])
                    nc.vector.tensor_copy(o1b[0:16, 4 + t0: 4 + t0 + 512], psB[:])

                if STAGES < 2:
                    nc.vector.tensor_copy(h2[:, 0:T], o1a[0:8, 4:4 + T])
                    nc.sync.dma_start(out_d[n], h2[:])
                    continue

                # stacked B operand: bstk rows 16g+r = o1b[r, g+j] (taps 0-7).
                # Two DMA triggers instead of eight: regular store to a DRAM
                # scratch, then one windowed load (3D source AP on DRAM).
                Wb = XW - 7
                bscr = bscrp.tile([16, XW], ddt)
                nc.sync.dma_start(bscr[:], o1b[0:16, :])
                nc.sync.dma_start(
                    bstk[:, 0:Wb],
                    make_ap(bscr[:], [[1, 8], [XW, 16], [1, Wb]]))

                # ---- stage 2: out2 (48ch padded to 64) = 9-tap conv of out1
                for tt in range(2):
                    t0 = tt * 512
                    ps2 = ps2p.tile([64, 512], dt.float32)
                    for dtp in range(9):
                        mm(ps2[:], w2a[:, dtp * 64: (dtp + 1) * 64],
                           o1a[:, t0 + dtp: t0 + dtp + 512], dtp == 0, False)
                    mm(ps2[:], w2bs[:], bstk[:, t0: t0 + 512], False, False)
                    mm(ps2[:], w2b8[:], o1b[:, t0 + 8: t0 + 8 + 512], False, True)
                    nc.vector.tensor_copy(o2s[0:64, 2 + t0: 2 + t0 + 512], ps2[:])

                if STAGES < 3:
                    nc.vector.tensor_copy(h2[:, 0:T], o2s[0:8, 2:2 + T])
                    nc.sync.dma_start(out_d[n], h2[:])
                    continue

                # o2 replica shifted by one tap (rows 64-127, DVE part-shift)
                nc.vector.tensor_copy(o2s[64:128, 0:O2W - 1], o2s[0:64, 1:O2W])

                # ---- stage 3: h1 (16ch padded to 32) = 5-tap conv, bn+lrelu
                for tt in range(2):
                    t0 = tt * 512
                    ps3 = ps3p.tile([32, 512], dt.float32)
                    mm(ps3[:], wc1s[:, 0:32], o2s[:, t0: t0 + 512], True, False)
                    mm(ps3[:], wc1s[:, 32:64], o2s[:, t0 + 2: t0 + 2 + 512],
                       False, False)
                    mm(ps3[:], wc1s[0:64, 64:96], o2s[0:64, t0 + 4: t0 + 4 + 512],
                       False, True)
                    nc.scalar.activation(o3s[0:32, 1 + t0: 1 + t0 + 512], ps3[:],
                                         LR, bias=sb3[:, 1:2], scale=sb3[:, 0:1],
                                         alpha=0.01)

                if STAGES < 4:
                    nc.vector.tensor_copy(h2[:, 0:T], o3s[0:8, 1:1 + T])
                    nc.sync.dma_start(out_d[n], h2[:])
                    continue

                # h1 replicas shifted by 1 and 2 taps (DVE part-shift)
                nc.vector.tensor_copy(o3s[32:64, 0:O3W - 1], o3s[0:32, 1:O3W])
                nc.vector.tensor_copy(o3s[64:96, 0:O3W - 2], o3s[0:32, 2:O3W])

                # ---- stage 4: h2 (8ch) = 3-tap conv, bn+lrelu
                for tt in range(2):
                    t0 = tt * 512
                    ps4 = ps4p.tile([8, 512], dt.float32)
                    mm(ps4[:], wc2s[:], o3s[:, t0: t0 + 512], True, True)
                    nc.scalar.activation(h2[:, t0: t0 + 512], ps4[:], LR,
                                         bias=sb4[:, 1:2], scale=sb4[:, 0:1],
                                         alpha=0.01)

                nc.sync.dma_start(out_d[n], h2[:])

    # TRN2 engine instructions accept a single sync-wait command, but Tile's
    # wait assignment can emit several (fresh DMA tick + PSUM-WAR tick, ...).
    # Legalize in two steps:
    #  1. matmuls: move extras onto the paired LDWEIGHTS (runs strictly
    #     earlier on the PE FIFO, so the stall point only moves up);
    #  2. anything still over the cap: hoist extras onto standalone
    #     EventSemaphore instructions inserted just before, on the same
    #     engine (stalls the sequencer where the instruction would have
    #     stalled anyway).
    for b in nc.m.functions[0].blocks:
        insts = list(b.instructions)
        for k, inst in enumerate(insts):
            if type(inst).__name__ != "InstMatmult":
                continue
            si = inst.sync_info
            if not si or len(si.on_wait) <= 1:
                continue
            prev = insts[k - 1]
            if type(prev).__name__ != "InstLdweights":
                continue
            psi = prev.sync_info
            prev.sync_info = mybir.SyncInfo(
                on_wait=list(si.on_wait[1:]) + (list(psi.on_wait) if psi else []),
                on_update=(list(psi.on_update) if psi else []))
            inst.sync_info = mybir.SyncInfo(
                on_wait=[si.on_wait[0]], on_update=list(si.on_update))

    esc = 0
    for b in nc.m.functions[0].blocks:
        insts = list(b.instructions)
        out = []
        changed = False
        for inst in insts:
            si = inst.sync_info
            nw = len(si.on_wait) if si and si.on_wait else 0
            if nw > 1 and type(inst).__name__ != "InstEventSemaphore":
                waits = list(si.on_wait)
                for w in waits[:-1]:
                    esc += 1
                    es = mybir.InstEventSemaphore(
                        name=f"ES-legal-{esc}", engine=inst.engine,
                        ins=[], outs=[], bass_nofuse=True)
                    es.sync_info = mybir.SyncInfo(on_wait=[w], on_update=[])
                    out.append(es)
                inst.sync_info = mybir.SyncInfo(
                    on_wait=[waits[-1]], on_update=list(si.on_update))
                changed = True
            out.append(inst)
        if changed:
            b.instructions = out

    return nc


def host_prep(inputs):
    poses = np.asarray(inputs["poses"], np.float32)
    fw = fold_weights(**{k: np.asarray(v) for k, v in inputs.items()
                         if k != "poses"})
    npdt = _np_dtype()

    Xp = np.zeros((N, 28, XW), np.float32)
    Xp[:, :27, 4:4 + T] = poses.transpose(0, 2, 1)
    Xp[:, 27, :] = 1.0
    Xp = np.ascontiguousarray(Xp.astype(npdt))

    W1t, W2t, Wc1t, Wc2t = fw["W1t"], fw["W2t"], fw["Wc1t"], fw["Wc2t"]

    w1s = np.zeros((126, 2 * 144), np.float32)
    for g in range(4):
        w1s[28 * g:28 * g + 28, 0:144] = W1t[g]
        w1s[28 * g:28 * g + 28, 144:288] = W1t[4 + g]
    w1s[112:126, 0:144] = W1t[8][0:14]
    w1s[112:126, 144:288] = W1t[8][14:28]

    w2ta = np.zeros((128, 9 * 64), np.float32)
    for dtp in range(9):
        w2ta[:, dtp * 64: dtp * 64 + 48] = W2t[dtp][:128]
    w2bs = np.zeros((128, 64), np.float32)
    for g in range(8):
        w2bs[16 * g:16 * g + 16, 0:48] = W2t[g][128:144]
    w2b8 = np.zeros((17, 64), np.float32)
    w2b8[:16, 0:48] = W2t[8][128:144]
    w2b8[16, 0:48] = W2t[4][144]        # bias row, applied once via ones row

    # s3 operand rows: 0-63 = out2(64pad) @ tap g, 64-127 = @ tap g+1
    wc1s = np.zeros((128, 3 * 32), np.float32)
    wc1s[0:48, 0:16] = Wc1t[0]; wc1s[64:112, 0:16] = Wc1t[1]
    wc1s[0:48, 32:48] = Wc1t[2]; wc1s[64:112, 32:48] = Wc1t[3]
    wc1s[0:48, 64:80] = Wc1t[4]

    # s4 operand rows: 0-31 = h1(32pad), 32-63 = h1@+1, 64-95 = h1@+2
    wc2s = np.zeros((96, 8), np.float32)
    wc2s[0:16] = Wc2t[0]
    wc2s[32:48] = Wc2t[1]
    wc2s[64:80] = Wc2t[2]

    sb3 = np.zeros((32, 2), np.float32)
    sb3[:16, 0] = fw["scale3"]; sb3[:16, 1] = fw["bias3"]
    sb3[16:, 0] = 1.0
    sb4 = np.stack([fw["scale4"], fw["bias4"]], axis=1).astype(np.float32)

    onesb = np.zeros((17, XW), np.float32)
    onesb[16, :] = 1.0

    common = dict(onesb=np.ascontiguousarray(onesb.astype(npdt)),
                  sb3=sb3, sb4=sb4,
                  w1s=np.ascontiguousarray(w1s.astype(npdt)),
                  w2ta=np.ascontiguousarray(w2ta.astype(npdt)),
                  w2bs=np.ascontiguousarray(w2bs.astype(npdt)),
                  w2b8=np.ascontiguousarray(w2b8.astype(npdt)),
                  wc1s=np.ascontiguousarray(wc1s.astype(npdt)),
                  wc2s=np.ascontiguousarray(wc2s.astype(npdt)))
    in_maps = []
    for c in range(N_CORES):
        m = dict(common)
        m["x"] = np.ascontiguousarray(Xp[c * NPC:(c + 1) * NPC])
        in_maps.append(m)
    return in_maps


def run(inputs, trace=False, tmpdir=None):
    global _BUILT
    from concourse import bass_utils
    if _BUILT is None:
        _BUILT = build_bass()
    nc = _BUILT
    in_maps = host_prep(inputs)
    res = bass_utils.run_bass_kernel_spmd(
        nc, in_maps, core_ids=list(range(N_CORES)), trace=trace,
        tmpdir=tmpdir)
    outs = [res.results[c]["out"] for c in range(N_CORES)]
    full = np.concatenate(outs, axis=0)          # (256, 8, 1024)
    return np.ascontiguousarray(full.transpose(0, 2, 1)).astype(np.float32), res


def kernel(**inputs) -> np.ndarray:
    out, _ = run(inputs, trace=False)
    return out


# revision 48
# speedup vs baseline: 1.3648x; 1.1509x over previous
"""AffEncoder Trainium2 kernel.

The network folds into 4 temporal-conv stages (channel-major):
  s1: K=28  (27 pose ch + ones row), M=144, 9 taps   (conv1 + A1 einsum folded)
  s2: K=145 (144 ch + ones row),     M=48,  9 taps   (gather + conv2 + A2 folded)
  s3: K=48, M=16, 5 taps, then Lrelu(scale*x+bias)   (convc1 + bn1 folded)
  s4: K=16, M=8,  3 taps, then Lrelu(scale*x+bias)   (convc2 + bn2 folded)

Sharding: pure data parallel, 32 batch elements per core across 8 cores.
Host does the (n,t,c)->(n,c,t) transposes + weight folding; the device runs
channel-major matmul pipelines.

Per batch element, per 512-col t-tile:
  s1: taps 0-3 / 4-7 pre-shifted into two 112-row operand stacks (one
      windowed DMA each); tap 8 = stack1 block 0 at window +8     -> 6 MM
  s2: 9 full-K passes (ch 0-127) + stacked B pass (ch 128-143 x 8 taps,
      one windowed SBUF->SBUF DMA) + tap-8/bias pass              -> 11 MM
  s3: K=128 double-tap passes on a partition-shifted replica      -> 3 MM
  s4: split K=32 (ACT-written rows) + K=64 (DVE replica rows)     -> 2 MM

TRN2 matmuls accept only ONE sync-wait, so each matmul's operands live in
tiles with a single producer: xs1/xs2/bstk are each written by exactly one
DMA instruction; o1a/o1b/o2s and the o3s replica rows are written only by
DVE; o3s rows 0-31 only by ACT.  Channels are padded (48->64, 16->32) so
partition-shifted replicas start 32-aligned.
"""
import os
import sys
import numpy as np

for _p in ("/opt/trn_rl_repo",):
    if _p not in sys.path and os.path.isdir(_p):
        sys.path.insert(0, _p)

import ml_dtypes  # noqa: E402

N_CORES = 8
N, T = 256, 1024
NPC = N // N_CORES
EPS = 1e-5
J, C, K1, K2, F1, F2 = 9, 3, 5, 3, 16, 16
NUM_PARTS, MAX_EDGES = 3, 3

XW = T + 12                 # x pad: 4 left, 8 right
O2W = T + 6                 # o2s pad: 2 left, 4 right
O3W = T + 4                 # o3s pad: 1 left, 3 right
DTYPE = os.environ.get("BASS_DTYPE", "bf16")  # bf16 | f32r | f32
STAGES = int(os.environ.get("STAGES", "4"))   # debug: truncate pipeline


def fold_weights(W1, b1, A1, W2, b2, A2, Wc1, bc1, bn1_w, bn1_b, bn1_m, bn1_v,
                 Wc2, bc2, bn2_w, bn2_b, bn2_m, bn2_v):
    W1 = np.asarray(W1, np.float64); A1 = np.asarray(A1, np.float64)
    W2 = np.asarray(W2, np.float64); A2 = np.asarray(A2, np.float64)

    W1r = W1[:, :, :, 0].reshape(K1, F1, C, 9)              # [k, c, ci, dt]
    W1t = np.zeros((9, 28, 144))
    W1t[:, :27, :] = np.einsum('kcid,kvw->dvicw', W1r, A1).reshape(9, 27, 144)
    beff1 = np.einsum('kc,kw->cw', np.asarray(b1, np.float64).reshape(K1, F1),
                      A1.sum(axis=1)).reshape(144)
    W1t[4, 27, :] = beff1

    W2r = W2[:, :, :, 0].reshape(K2, F2, F1, MAX_EDGES, 9)  # [k2, c2, c, e, dt]
    W2t = np.zeros((9, 145, 48))
    W2t[:, :144, :] = np.einsum('kbced,kpq->dcpebq', W2r, A2).reshape(9, 144, 48)
    beff2 = np.einsum('kb,kq->bq', np.asarray(b2, np.float64).reshape(K2, F2),
                      A2.sum(axis=1)).reshape(48)
    W2t[4, 144, :] = beff2

    Wc1t = np.asarray(Wc1, np.float64).transpose(2, 1, 0)   # [dt, m2, c3]
    scale3 = np.asarray(bn1_w, np.float64) / np.sqrt(np.asarray(bn1_v, np.float64) + EPS)
    bias3 = scale3 * np.asarray(bc1, np.float64) + (np.asarray(bn1_b, np.float64)
            - np.asarray(bn1_m, np.float64) * scale3)
    Wc2t = np.asarray(Wc2, np.float64).transpose(2, 1, 0)   # [dt, c3, c4]
    scale4 = np.asarray(bn2_w, np.float64) / np.sqrt(np.asarray(bn2_v, np.float64) + EPS)
    bias4 = scale4 * np.asarray(bc2, np.float64) + (np.asarray(bn2_b, np.float64)
            - np.asarray(bn2_m, np.float64) * scale4)
    return dict(W1t=W1t, W2t=W2t, Wc1t=Wc1t, scale3=scale3, bias3=bias3,
                Wc2t=Wc2t, scale4=scale4, bias4=bias4)


def _np_dtype():
    return ml_dtypes.bfloat16 if DTYPE == "bf16" else np.float32


_BUILT = None


def build_bass():
    import concourse.bass as bass
    import concourse.mybir as mybir
    from concourse import tile
    from concourse.tile import add_dep_helper
    from bass_rust import AP

    dt = mybir.dt
    if DTYPE == "bf16":
        ddt, mdt = dt.bfloat16, dt.bfloat16
    elif DTYPE == "f32r":
        ddt, mdt = dt.float32, dt.float32r
    else:
        ddt, mdt = dt.float32, dt.float32

    nc = bass.Bass("TRN2", target_bir_lowering=False, debug=False,
                   num_devices=N_CORES)

    x_d = nc.dram_tensor("x", (NPC, 28, XW), ddt, kind="ExternalInput")
    w1s_d = nc.dram_tensor("w1s", (126, 2 * 144), ddt, kind="ExternalInput")
    w2a_d = nc.dram_tensor("w2ta", (128, 9 * 64), ddt, kind="ExternalInput")
    w2bs_d = nc.dram_tensor("w2bs", (128, 64), ddt, kind="ExternalInput")
    w2b8_d = nc.dram_tensor("w2b8", (17, 64), ddt, kind="ExternalInput")
    wc1s_d = nc.dram_tensor("wc1s", (128, 3 * 32), ddt, kind="ExternalInput")
    wc2s_d = nc.dram_tensor("wc2s", (96, 8), ddt, kind="ExternalInput")
    sb3_d = nc.dram_tensor("sb3", (32, 2), dt.float32, kind="ExternalInput")
    sb4_d = nc.dram_tensor("sb4", (8, 2), dt.float32, kind="ExternalInput")
    onesb_d = nc.dram_tensor("onesb", (17, XW), ddt, kind="ExternalInput")
    out_d = nc.dram_tensor("out", (NPC, 8, T), dt.float32, kind="ExternalOutput")

    LR = (mybir.ActivationFunctionType.Relu
          if os.environ.get("SIM_ACT") == "relu"
          else mybir.ActivationFunctionType.Lrelu)

    def mm(out, lhsT, rhs, start, stop):
        return nc.tensor.matmul(
            out, lhsT.bitcast(mdt) if mdt != ddt else lhsT,
            rhs.bitcast(mdt) if mdt != ddt else rhs,
            start=start, stop=stop)

    def make_ap(base, ap_list, extra_offset=0):
        return AP(tensor=base.tensor, offset=base.offset + extra_offset,
                  ap=ap_list, const_val=base.const_val,
                  runtime_checks=base.runtime_checks)

    with tile.TileContext(nc) as tc:
        with (
            tc.tile_pool(name="wpool", bufs=1) as wpool,
            tc.tile_pool(name="xpool", bufs=6) as xpool,
            tc.tile_pool(name="o1a", bufs=3) as o1ap,
            tc.tile_pool(name="o1b", bufs=3) as o1bp,
            tc.tile_pool(name="o2", bufs=3) as o2p,
            tc.tile_pool(name="o3", bufs=3) as o3p,
            tc.tile_pool(name="h2", bufs=3) as h2p,
            tc.tile_pool(name="bscr", bufs=3, space="DRAM") as bscrp,
            tc.tile_pool(name="ps1a", bufs=2, space="PSUM") as ps1ap,
            tc.tile_pool(name="ps1b", bufs=2, space="PSUM") as ps1bp,
            tc.tile_pool(name="ps2", bufs=2, space="PSUM") as ps2p,
            tc.tile_pool(name="ps3", bufs=1, space="PSUM") as ps3p,
            tc.tile_pool(name="ps4", bufs=1, space="PSUM") as ps4p,
        ):
            w1s = wpool.tile([126, 2 * 144], ddt)
            w2a = wpool.tile([128, 9 * 64], ddt)
            w2bs = wpool.tile([128, 64], ddt)
            w2b8 = wpool.tile([17, 64], ddt)
            wc1s = wpool.tile([128, 3 * 32], ddt)
            wc2s = wpool.tile([96, 8], ddt)
            sb3 = wpool.tile([32, 2], dt.float32)
            sb4 = wpool.tile([8, 2], dt.float32)
            zt = wpool.tile([128, 8], dt.float32)
            onesB = wpool.tile([17, XW], ddt)
            nc.gpsimd.memset(zt[:], 0.0)
            for tile_, dram in ((w1s, w1s_d), (w2a, w2a_d),
                                (w2bs, w2bs_d), (w2b8, w2b8_d), (wc1s, wc1s_d),
                                (wc2s, wc2s_d), (sb3, sb3_d), (sb4, sb4_d),
                                (onesB, onesb_d)):
                nc.sync.dma_start(tile_[:], dram[:])

            for n in range(NPC):
                # --- stage-1 operand stacks: windowed DMA for the 4 tap
                # blocks + a half of tap 8 in rows 112-125.
                # xs1 rows 28g+r = x[n][r, g+j]   (taps 0-3), rows 112-125 =
                #     x[n][0:14, 8+j] (tap 8, channels 0-13)
                # xs2 rows 28g+r = x[n][r, 4+g+j] (taps 4-7), rows 112-125 =
                #     x[n][14:28, 8+j] (tap 8, channels 14-27)
                xs1 = xpool.tile([126, XW], ddt, tag="xs")
                xs2 = xpool.tile([126, XW], ddt, tag="xs")
                W1w = XW - 3
                W2w = XW - 7
                W8w = XW - 8
                xn = x_d[n]
                nc.sync.dma_start(
                    xs1[0:112, 0:W1w],
                    make_ap(xn, [[1, 4], [XW, 28], [1, W1w]]))
                nc.sync.dma_start(
                    xs2[0:112, 0:W2w],
                    make_ap(xn, [[1, 4], [XW, 28], [1, W2w]], extra_offset=4))
                nc.sync.dma_start(xs1[112:126, 0:W8w], xn[0:14, 8:XW])
                nc.sync.dma_start(xs2[112:126, 0:W8w], xn[14:28, 8:XW])

                o1a = o1ap.tile([128, XW], ddt)
                o1b = o1bp.tile([17, XW], ddt)
                bstk = o1ap.tile([128, XW], ddt, tag="bstk")
                o2s = o2p.tile([128, O2W], ddt)
                o3s = o3p.tile([96, O3W], ddt)
                h2 = h2p.tile([8, T], dt.float32)

                # halo zeroing + ones row (same engine as the tile's writer)
                nc.vector.tensor_copy(o1a[:, 0:4], zt[:, 0:4])
                nc.vector.tensor_copy(o1a[:, T + 4:XW], zt[:, 0:8])
                # zeros rows 0-15 (halos) + ones row 16, in one aligned copy;
                # evictions overwrite the data region afterwards
                nc.vector.tensor_copy(o1b[:], onesB[:])
                nc.vector.tensor_copy(o2s[0:64, 0:2], zt[0:64, 0:2])
                nc.vector.tensor_copy(o2s[0:64, T + 2:O2W], zt[0:64, 0:4])
                nc.scalar.copy(o3s[0:32, 0:1], zt[0:32, 0:1])
                nc.scalar.copy(o3s[0:32, T + 1:O3W], zt[0:32, 0:3])

                # ---- stage 1: out1 (144ch) = 9-tap conv of x (28ch)
                for tt in range(2):
                    t0 = tt * 512
                    psA = ps1ap.tile([128, 512], dt.float32)
                    psB = ps1bp.tile([16, 512], dt.float32)
                    r1 = xs1[:, t0: t0 + 512]
                    r2 = xs2[:, t0: t0 + 512]
                    mm(psA[:], w1s[:, 0:128], r1, True, False)
                    mm(psA[:], w1s[:, 144:272], r2, False, True)
                    mm(psB[:], w1s[:, 128:144], r1, True, False)
                    mm(psB[:], w1s[:, 272:288], r2, False, True)
                    nc.vector.tensor_copy(o1a[:, 4 + t0: 4 + t0 + 512], psA[:])
                    nc.vector.tensor_copy(o1b[0:16, 4 + t0: 4 + t0 + 512], psB[:])

                if STAGES < 2:
                    nc.vector.tensor_copy(h2[:, 0:T], o1a[0:8, 4:4 + T])
                    nc.sync.dma_start(out_d[n], h2[:])
                    continue

                # stacked B operand: bstk rows 16g+r = o1b[r, g+j] (taps 0-7).
                # Two DMA triggers instead of eight: regular store to a DRAM
                # scratch, then one windowed load (3D source AP on DRAM).
                Wb = XW - 7
                bscr = bscrp.tile([16, XW], ddt)
                nc.sync.dma_start(bscr[:], o1b[0:16, :])
                nc.sync.dma_start(
                    bstk[:, 0:Wb],
                    make_ap(bscr[:], [[1, 8], [XW, 16], [1, Wb]]))

                # ---- stage 2: out2 (48ch padded to 64) = 9-tap conv of out1
                for tt in range(2):
                    t0 = tt * 512
                    ps2 = ps2p.tile([64, 512], dt.float32)
                    for dtp in range(9):
                        mm(ps2[:], w2a[:, dtp * 64: (dtp + 1) * 64],
                           o1a[:, t0 + dtp: t0 + dtp + 512], dtp == 0, False)
                    mm(ps2[:], w2bs[:], bstk[:, t0: t0 + 512], False, False)
                    mm(ps2[:], w2b8[:], o1b[:, t0 + 8: t0 + 8 + 512], False, True)
                    nc.vector.tensor_copy(o2s[0:64, 2 + t0: 2 + t0 + 512], ps2[:])

                if STAGES < 3:
                    nc.vector.tensor_copy(h2[:, 0:T], o2s[0:8, 2:2 + T])
                    nc.sync.dma_start(out_d[n], h2[:])
                    continue

                # o2 replica shifted by one tap (rows 64-127, DVE part-shift)
                nc.vector.tensor_copy(o2s[64:128, 0:O2W - 1], o2s[0:64, 1:O2W])

                # ---- stage 3: h1 (16ch padded to 32) = 5-tap conv, bn+lrelu
                for tt in range(2):
                    t0 = tt * 512
                    ps3 = ps3p.tile([32, 512], dt.float32)
                    mm(ps3[:], wc1s[:, 0:32], o2s[:, t0: t0 + 512], True, False)
                    mm(ps3[:], wc1s[:, 32:64], o2s[:, t0 + 2: t0 + 2 + 512],
                       False, False)
                    mm(ps3[:], wc1s[0:64, 64:96], o2s[0:64, t0 + 4: t0 + 4 + 512],
                       False, True)
                    nc.scalar.activation(o3s[0:32, 1 + t0: 1 + t0 + 512], ps3[:],
                                         LR, bias=sb3[:, 1:2], scale=sb3[:, 0:1],
                                         alpha=0.01)

                if STAGES < 4:
                    nc.vector.tensor_copy(h2[:, 0:T], o3s[0:8, 1:1 + T])
                    nc.sync.dma_start(out_d[n], h2[:])
                    continue

                # h1 replicas shifted by 1 and 2 taps (DVE part-shift)
                nc.vector.tensor_copy(o3s[32:64, 0:O3W - 1], o3s[0:32, 1:O3W])
                nc.vector.tensor_copy(o3s[64:96, 0:O3W - 2], o3s[0:32, 2:O3W])

                # ---- stage 4: h2 (8ch) = 3-tap conv, bn+lrelu
                for tt in range(2):
                    t0 = tt * 512
                    ps4 = ps4p.tile([8, 512], dt.float32)
                    mm(ps4[:], wc2s[:], o3s[:, t0: t0 + 512], True, True)
                    nc.scalar.activation(h2[:, t0: t0 + 512], ps4[:], LR,
                                         bias=sb4[:, 1:2], scale=sb4[:, 0:1],
                                         alpha=0.01)

                nc.sync.dma_start(out_d[n], h2[:])

    # TRN2 engine instructions accept a single sync-wait command, but Tile's
    # wait assignment can emit several (fresh DMA tick + PSUM-WAR tick, ...).
    # Legalize in two steps:
    #  1. matmuls: move extras onto the paired LDWEIGHTS (runs strictly
    #     earlier on the PE FIFO, so the stall point only moves up);
    #  2. anything still over the cap: hoist extras onto standalone
    #     EventSemaphore instructions inserted just before, on the same
    #     engine (stalls the sequencer where the instruction would have
    #     stalled anyway).
    for b in nc.m.functions[0].blocks:
        insts = list(b.instructions)
        for k, inst in enumerate(insts):
            if type(inst).__name__ != "InstMatmult":
                continue
            si = inst.sync_info
            if not si or len(si.on_wait) <= 1:
                continue
            prev = insts[k - 1]
            if type(prev).__name__ != "InstLdweights":
                continue
            psi = prev.sync_info
            prev.sync_info = mybir.SyncInfo(
                on_wait=list(si.on_wait[1:]) + (list(psi.on_wait) if psi else []),
                on_update=(list(psi.on_update) if psi else []))
            inst.sync_info = mybir.SyncInfo(
                on_wait=[si.on_wait[0]], on_update=list(si.on_update))

    esc = 0
    for b in nc.m.functions[0].blocks:
        insts = list(b.instructions)
        out = []
        changed = False
        for inst in insts:
            si = inst.sync_info
            nw = len(si.on_wait) if si and si.on_wait else 0
            if nw > 1 and type(inst).__name__ != "InstEventSemaphore":
                waits = list(si.on_wait)
                for w in waits[:-1]:
                    esc += 1
                    es = mybir.InstEventSemaphore(
                        name=f"ES-legal-{esc}", engine=inst.engine,
                        ins=[], outs=[], bass_nofuse=True)
                    es.sync_info = mybir.SyncInfo(on_wait=[w], on_update=[])
                    out.append(es)
                inst.sync_info = mybir.SyncInfo(
                    on_wait=[waits[-1]], on_update=list(si.on_update))
                changed = True
            out.append(inst)
        if changed:
            b.instructions = out

    return nc


def host_prep(inputs):
    poses = np.asarray(inputs["poses"], np.float32)
    fw = fold_weights(**{k: np.asarray(v) for k, v in inputs.items()
                         if k != "poses"})
    npdt = _np_dtype()

    Xp = np.zeros((N, 28, XW), np.float32)
    Xp[:, :27, 4:4 + T] = poses.transpose(0, 2, 1)
    Xp[:, 27, :] = 1.0
    Xp = np.ascontiguousarray(Xp.astype(npdt))

    W1t, W2t, Wc1t, Wc2t = fw["W1t"], fw["W2t"], fw["Wc1t"], fw["Wc2t"]

    w1s = np.zeros((126, 2 * 144), np.float32)
    for g in range(4):
        w1s[28 * g:28 * g + 28, 0:144] = W1t[g]
        w1s[28 * g:28 * g + 28, 144:288] = W1t[4 + g]
    w1s[112:126, 0:144] = W1t[8][0:14]
    w1s[112:126, 144:288] = W1t[8][14:28]

    w2ta = np.zeros((128, 9 * 64), np.float32)
    for dtp in range(9):
        w2ta[:, dtp * 64: dtp * 64 + 48] = W2t[dtp][:128]
    w2bs = np.zeros((128, 64), np.float32)
    for g in range(8):
        w2bs[16 * g:16 * g + 16, 0:48] = W2t[g][128:144]
    w2b8 = np.zeros((17, 64), np.float32)
    w2b8[:16, 0:48] = W2t[8][128:144]
    w2b8[16, 0:48] = W2t[4][144]        # bias row, applied once via ones row

    # s3 operand rows: 0-63 = out2(64pad) @ tap g, 64-127 = @ tap g+1
    wc1s = np.zeros((128, 3 * 32), np.float32)
    wc1s[0:48, 0:16] = Wc1t[0]; wc1s[64:112, 0:16] = Wc1t[1]
    wc1s[0:48, 32:48] = Wc1t[2]; wc1s[64:112, 32:48] = Wc1t[3]
    wc1s[0:48, 64:80] = Wc1t[4]

    # s4 operand rows: 0-31 = h1(32pad), 32-63 = h1@+1, 64-95 = h1@+2
    wc2s = np.zeros((96, 8), np.float32)
    wc2s[0:16] = Wc2t[0]
    wc2s[32:48] = Wc2t[1]
    wc2s[64:80] = Wc2t[2]

    sb3 = np.zeros((32, 2), np.float32)
    sb3[:16, 0] = fw["scale3"]; sb3[:16, 1] = fw["bias3"]
    sb3[16:, 0] = 1.0
    sb4 = np.stack([fw["scale4"], fw["bias4"]], axis=1).astype(np.float32)

    onesb = np.zeros((17, XW), np.float32)
    onesb[16, :] = 1.0

    common = dict(onesb=np.ascontiguousarray(onesb.astype(npdt)),
                  sb3=sb3, sb4=sb4,
                  w1s=np.ascontiguousarray(w1s.astype(npdt)),
                  w2ta=np.ascontiguousarray(w2ta.astype(npdt)),
                  w2bs=np.ascontiguousarray(w2bs.astype(npdt)),
                  w2b8=np.ascontiguousarray(w2b8.astype(npdt)),
                  wc1s=np.ascontiguousarray(wc1s.astype(npdt)),
                  wc2s=np.ascontiguousarray(wc2s.astype(npdt)))
    in_maps = []
    for c in range(N_CORES):
        m = dict(common)
        m["x"] = np.ascontiguousarray(Xp[c * NPC:(c + 1) * NPC])
        in_maps.append(m)
    return in_maps


def run(inputs, trace=False, tmpdir=None):
    global _BUILT
    from concourse import bass_utils
    if _BUILT is None:
        _BUILT = build_bass()
    nc = _BUILT
    in_maps = host_prep(inputs)
    res = bass_utils.run_bass_kernel_spmd(
        nc, in_maps, core_ids=list(range(N_CORES)), trace=trace,
        tmpdir=tmpdir)
    outs = [res.results[c]["out"] for c in range(N_CORES)]
    full = np.concatenate(outs, axis=0)          # (256, 8, 1024)
    return np.ascontiguousarray(full.transpose(0, 2, 1)).astype(np.float32), res


def kernel(**inputs) -> np.ndarray:
    out, _ = run(inputs, trace=False)
    return out
